# revision 6
# baseline (speedup 1.0000x reference)
"""GCN (5-layer) + global mean pool + MLP head on 8 trn2 NeuronCores. v4.

Dest-sharded with per-layer AllGather straight into a contiguous shared
y-table; tight-packed dma_gather token streams.

  - Factorized GCN norm: with dis = rsqrt(deg), y = dis * (h @ W):
      h'[v] = relu(dis[v] * (sum_{u->v} y[u] + y[v]) + b)
  - Nodes sharded across 8 cores (12500 each, padded to 12544 = 98 tiles
    of 128), laid out contiguously: node at canonical position i of core c
    sits at table row c*12544 + i.  The per-layer AllGather writes the
    [100352, 16] f32 table directly (Shared scratchpad, double-buffered) --
    no copies.
  - dma_gather needs a 256B source stride, so edges are split in 4 streams
    by table row % 4: stream q gathers 64B rows at byte offset q*64 within
    each 256B group (in_ap offset q*16 f32, elem_step 64 f32, idx = row//4
    which fits int16).
  - Per (core, stream) destination orderings are degree-sorted; columns
    packed tile-major with K_t slots per tile (K_t = cross-core max
    in-tile degree).  Reduces run per K-run with the final halving step
    writing straight into the stream slab; 4 realign gathers bring slabs
    back to canonical (total-degree) order.
"""
import inspect
import re

import numpy as np

import concourse.bass as bass
import concourse.bacc as bacc
import concourse.tile as tile
import concourse.mybir as mybir
from concourse.bass2jax import run_bass_via_pjrt
from concourse.masks import make_identity

F32 = mybir.dt.float32
I16 = mybir.dt.int16
AL = mybir.AluOpType

N_NODES = 100000
N_EDGES = 3200000
N_GRAPHS = 1000
HID = 16
C = 8                    # cores
NPC = N_NODES // C       # 12500 nodes per core
P = 128
TILES = 98               # ceil(12500/128)
NPAD = TILES * P         # 12544
TROWS = C * NPAD         # 100352 table rows
GROWS = TROWS // 4       # 25088 gather grid rows (256B each)
ROW = 64                 # rslab row stride in f32 (256B)
GPC = N_GRAPHS // C      # 125 graphs per core
ICOLS = 32               # max token columns per gather instruction (4096 idx)
BUFCOLS = 256            # gather ring-buffer columns (16KB/partition)
PAD_IDX = 12500 // 4     # 3125: core-0 rows 12500..12503 are zeroed pads
RRELU_SLOPE = (1.0 / 8.0 + 1.0 / 3.0) / 2.0


def _make_patched_dma_gather():
    """dma_gather with the elem_size%256 assert dropped (the 256B constraint
    is on the row stride; 64B payloads from a 256B-strided table work)."""
    src = inspect.getsource(bass.BassGpSimd.dma_gather)
    src = src.replace(
        "assert (\n            elem_size_bytes > 0 and elem_size_bytes % 256 == 0\n        )  # transpose restriction",
        "assert elem_size_bytes > 0")
    src = re.sub(r"^    def dma_gather", "def dma_gather_patched", src, flags=re.M)
    src = "\n".join(l[4:] if l.startswith("    ") else l for l in src.splitlines())
    ns = dict(bass.__dict__)
    exec(src, ns)
    return ns["dma_gather_patched"]


_dma_gather = _make_patched_dma_gather()


def _wrap_idx(tokens):
    """[128, cols] token array (token i at (i%128, i//128)) -> [128, cols*8]
    int16 wrapped index layout (idx i at (i%16, i//16), replicated x8)."""
    p, cols = tokens.shape
    assert p == P
    flat = tokens.T.reshape(-1)                  # i = col*128 + p
    w16 = flat.reshape(-1, 16).T                 # [16, cols*8]
    return np.tile(w16, (8, 1)).astype(np.int16)


def _runs_of(ks):
    runs = []
    i = 0
    while i < len(ks):
        j = i
        while j < len(ks) and ks[j] == ks[i]:
            j += 1
        runs.append((int(ks[i]), i, j - i))
        i = j
    return runs


def _preprocess(x, edge_index, batch):
    """Build per-core device inputs + shared (cross-core identical) plan."""
    src = np.asarray(edge_index[0], dtype=np.int64)
    dst = np.asarray(edge_index[1], dtype=np.int64)
    batch = np.asarray(batch, dtype=np.int64)
    x = np.asarray(x, dtype=np.float32)

    deg = np.bincount(dst, minlength=N_NODES).astype(np.float32) + 1.0
    tot_in = np.bincount(dst, minlength=N_NODES)

    dcore = dst // NPC

    # canonical order per core: total in-degree sort; table row of each node
    perm_tot = []
    row_of_node = np.empty(N_NODES, dtype=np.int64)
    for c in range(C):
        lo = c * NPC
        order = np.argsort(-tot_in[lo:lo + NPC], kind="stable")
        perm_tot.append(order)
        row_of_node[lo + order] = c * NPAD + np.arange(NPC)

    # stream of an edge = source table row % 4; gather idx = row // 4
    srow = row_of_node[src]
    rng = srow % 4
    sidx = srow // 4

    key = dst * 4 + rng
    cnt4 = np.bincount(key, minlength=N_NODES * 4).reshape(N_NODES, 4)

    # per-(core, stream) destination orderings
    perm = [[None] * 4 for _ in range(C)]
    rank_in_perm = [[None] * 4 for _ in range(C)]
    for c in range(C):
        lo = c * NPC
        for r in range(4):
            d = cnt4[lo:lo + NPC, r]
            order = np.argsort(-d, kind="stable")
            perm[c][r] = order
            rk = np.empty(NPC, dtype=np.int64)
            rk[order] = np.arange(NPC)
            rank_in_perm[c][r] = rk

    # K per (stream, tile): cross-core max of tile-max degree (>= 1)
    Ks = np.zeros((4, TILES), dtype=np.int64)
    for c in range(C):
        for r in range(4):
            d_sorted = cnt4[c * NPC:(c + 1) * NPC, r][perm[c][r]]
            d_pad = np.concatenate([d_sorted, np.zeros(NPAD - NPC, np.int64)])
            tile_max = d_pad.reshape(TILES, P).max(axis=1)
            Ks[r] = np.maximum(Ks[r], tile_max)
    Ks = np.maximum(Ks, 1)

    col_base = np.zeros((4, TILES), dtype=np.int64)
    gcols = np.zeros(4, dtype=np.int64)
    for r in range(4):
        col_base[r] = np.concatenate([[0], np.cumsum(Ks[r])[:-1]])
        gcols[r] = int(Ks[r].sum())

    # buffer plan per stream: whole tiles, <= BUFCOLS columns each
    bufs = [[] for _ in range(4)]
    for r in range(4):
        t = 0
        while t < TILES:
            t0 = t
            cols = 0
            while t < TILES and cols + Ks[r][t] <= BUFCOLS:
                cols += int(Ks[r][t])
                t += 1
            assert t > t0
            runs = _runs_of(list(Ks[r][t0:t]))
            bufs[r].append((t0, t - t0, int(col_base[r][t0]), cols, runs))

    # token arrays per core per stream: [128, gcols_r] table grid indices
    tok = [[np.full((P, int(gcols[r])), PAD_IDX, dtype=np.int64)
            for r in range(4)] for _ in range(C)]
    erank = np.empty(N_EDGES, dtype=np.int64)
    for c in range(C):
        m = dcore == c
        for r in range(4):
            mm = m & (rng == r)
            erank[mm] = rank_in_perm[c][r][dst[mm] - c * NPC]
    order = np.lexsort((erank, rng, dcore))
    so_dcore, so_rng, so_rank = dcore[order], rng[order], erank[order]
    gkey = (so_dcore * 4 + so_rng) * NPC + so_rank
    starts = np.concatenate([[True], gkey[1:] != gkey[:-1]])
    gidx = np.cumsum(starts) - 1
    first = np.flatnonzero(starts)
    slot = np.arange(len(order)) - first[gidx]
    so_sidx = sidx[order]
    for c in range(C):
        m = so_dcore == c
        for r in range(4):
            mm = m & (so_rng == r)
            rk = so_rank[mm]
            tl = rk // P
            col = col_base[r][tl] + slot[mm]
            tok[c][r][rk % P, col] = so_sidx[mm]

    # align-gather indices: canonical position i -> rank in perm_r
    align_idx = [[None] * 4 for _ in range(C)]
    for c in range(C):
        for r in range(4):
            ai = np.arange(NPAD, dtype=np.int64)
            ai[:NPC] = rank_in_perm[c][r][perm_tot[c]]
            align_idx[c][r] = ai

    # pooling: graph g -> core g // GPC; single 125-partition tile per stream
    g_of_node = batch
    node_rows = row_of_node
    node_rng = node_rows % 4
    node_idx = node_rows // 4
    pkey = (g_of_node * 4 + node_rng)
    pcnt = np.bincount(pkey, minlength=N_GRAPHS * 4).reshape(N_GRAPHS, 4)
    Kp = [max(1, int(pcnt[:, r].max())) for r in range(4)]

    ptok = [[np.full((P, Kp[r]), PAD_IDX, dtype=np.int64)
             for r in range(4)] for _ in range(C)]
    porder = np.lexsort((g_of_node, node_rng))
    po_g, po_rng = g_of_node[porder], node_rng[porder]
    pk = po_g * 4 + po_rng
    pstarts = np.concatenate([[True], pk[1:] != pk[:-1]])
    pgidx = np.cumsum(pstarts) - 1
    pfirst = np.flatnonzero(pstarts)
    pslot = np.arange(len(porder)) - pfirst[pgidx]
    po_idx = node_idx[porder]
    for r in range(4):
        m = po_rng == r
        g = po_g[m]
        c = g // GPC
        part = g % GPC
        sl = pslot[m]
        for cc in range(C):
            mm = c == cc
            ptok[cc][r][part[mm], sl[mm]] = po_idx[m][mm]

    cnt_graph = np.bincount(batch, minlength=N_GRAPHS).astype(np.float32)
    cnt_graph = np.maximum(cnt_graph, 1.0)

    per_core = []
    for c in range(C):
        lo = c * NPC
        sigma = perm_tot[c]
        nodes_sigma = lo + sigma
        deg_pad = np.ones(NPAD, dtype=np.float32)
        deg_pad[:NPC] = deg[nodes_sigma]
        deg_tiles = deg_pad.reshape(TILES, P).T.copy()  # [128, 98]

        nblk = (TILES + 7) // 8
        xp = np.zeros((nblk * 8 * P, 4), dtype=np.float32)
        xp[:NPC] = x[nodes_sigma]
        xt = xp.reshape(nblk, 8, P, 4).transpose(1, 3, 0, 2).reshape(32, nblk * P).copy()

        gather_w = np.concatenate(
            [_wrap_idx(tok[c][r]) for r in range(4)], axis=1)
        align_w_parts = []
        for r in range(4):
            a = np.zeros((P, P), dtype=np.int64)
            a[:, :TILES] = align_idx[c][r].reshape(TILES, P).T
            align_w_parts.append(_wrap_idx(a))
        align_w = np.concatenate(align_w_parts, axis=1)
        pool_w = np.concatenate(
            [_wrap_idx(ptok[c][r]) for r in range(4)], axis=1)

        cnt_c = np.ones((P, 1), dtype=np.float32)
        cnt_c[:GPC, 0] = cnt_graph[c * GPC:(c + 1) * GPC]

        per_core.append(dict(
            deg_tiles=deg_tiles, xt=xt, gather_w=gather_w,
            align_w=align_w, pool_w=pool_w, cnt=cnt_c))

    plan = dict(Ks=Ks, col_base=col_base, gcols=gcols, bufs=bufs, Kp=Kp)
    return per_core, plan


def _build_program(plan, reps=1, mode='full'):
    Ks = plan["Ks"]
    gcols = plan["gcols"]
    bufs = plan["bufs"]
    Kp = plan["Kp"]

    gather_wcols = int(sum(gcols)) * 8
    align_wcols = 4 * P * 8
    pool_wcols = int(sum(Kp)) * 8

    nc = bacc.Bacc(None, target_bir_lowering=False, num_devices=C,
                   num_swdge_queues=4)

    deg_in = nc.dram_tensor("deg_tiles", [P, TILES], F32, kind="ExternalInput")
    NBLK = (TILES + 7) // 8
    xt_in = nc.dram_tensor("xt", [32, NBLK * P], F32, kind="ExternalInput")
    gw_in = nc.dram_tensor("gather_w", [P, gather_wcols], I16, kind="ExternalInput")
    aw_in = nc.dram_tensor("align_w", [P, align_wcols], I16, kind="ExternalInput")
    pw_in = nc.dram_tensor("pool_w", [P, pool_wcols], I16, kind="ExternalInput")
    cnt_in = nc.dram_tensor("cnt", [P, 1], F32, kind="ExternalInput")
    ws_in = {}
    ws_in["W1"] = nc.dram_tensor("W1", [32, P], F32, kind="ExternalInput")
    for i in range(2, 6):
        ws_in[f"W{i}"] = nc.dram_tensor(f"W{i}", [P, P], F32, kind="ExternalInput")
    b_in = nc.dram_tensor("bs", [P, 5 * HID], F32, kind="ExternalInput")
    l1w_in = nc.dram_tensor("lin1_w", [HID, HID], F32, kind="ExternalInput")
    l1b_in = nc.dram_tensor("lin1_b", [P, HID], F32, kind="ExternalInput")
    l2w_in = nc.dram_tensor("lin2_w", [HID, 1], F32, kind="ExternalInput")
    l2b_in = nc.dram_tensor("lin2_b", [P, 1], F32, kind="ExternalInput")
    out_t = nc.dram_tensor("out", [P, 1], F32, kind="ExternalOutput")

    # internal DRAM: double-buffered shared y-table + realign scratch
    tblA = nc.dram_tensor("tblA", [TROWS, HID], F32, addr_space="Shared")
    tblB = nc.dram_tensor("tblB", [TROWS, HID], F32, addr_space="Shared")
    tbl = [tblA, tblB]
    rslab = nc.dram_tensor("rslab", [4 * NPAD, ROW], F32)
    ag_in = nc.dram_tensor("ag_in", [NPAD, HID], F32)

    core_id = nc.partition_id_tensor  # noqa: F841

    with tile.TileContext(nc) as tc:
        import contextlib
        with contextlib.ExitStack() as ctx:
            sbp = ctx.enter_context(tc.tile_pool(name="persist", bufs=1))
            gp = ctx.enter_context(tc.tile_pool(name="g", bufs=3))
            smp = ctx.enter_context(tc.tile_pool(name="sm", bufs=4))
            psp = ctx.enter_context(tc.tile_pool(name="ps", bufs=3, space="PSUM"))
            pst = ctx.enter_context(tc.tile_pool(name="pst", bufs=3, space="PSUM"))

            idx_g = sbp.tile([P, gather_wcols], I16)
            idx_a = sbp.tile([P, align_wcols], I16)
            idx_p = sbp.tile([P, pool_wcols], I16)
            nc.sync.dma_start(idx_g[:], gw_in[:])
            nc.sync.dma_start(idx_a[:], aw_in[:])
            nc.sync.dma_start(idx_p[:], pw_in[:])

            deg_sb = sbp.tile([P, TILES], F32)
            nc.sync.dma_start(deg_sb[:], deg_in[:])
            dis_sb = sbp.tile([P, TILES], F32)
            nc.scalar.activation(out=dis_sb[:], in_=deg_sb[:],
                                 func=mybir.ActivationFunctionType.Sqrt)
            nc.vector.reciprocal(out=dis_sb[:], in_=dis_sb[:])

            xt_sb = sbp.tile([32, NBLK * P], F32)
            nc.sync.dma_start(xt_sb[:], xt_in[:])

            w_sb = {}
            w_sb[1] = sbp.tile([32, P], F32, tag="w1", name="w1")
            nc.sync.dma_start(w_sb[1][:], ws_in["W1"][:])
            for i in range(2, 6):
                w_sb[i] = sbp.tile([P, P], F32, tag=f"w{i}", name=f"w{i}")
                nc.sync.dma_start(w_sb[i][:], ws_in[f"W{i}"][:])
            b_sb = sbp.tile([P, 5 * HID], F32)
            nc.sync.dma_start(b_sb[:], b_in[:])
            l1w_sb = sbp.tile([HID, HID], F32)
            nc.sync.dma_start(l1w_sb[:], l1w_in[:])
            l1b_sb = sbp.tile([P, HID], F32)
            nc.sync.dma_start(l1b_sb[:], l1b_in[:])
            l2w_sb = sbp.tile([HID, 1], F32)
            nc.sync.dma_start(l2w_sb[:], l2w_in[:])
            l2b_sb = sbp.tile([P, 1], F32)
            nc.sync.dma_start(l2b_sb[:], l2b_in[:])
            cnt_sb = sbp.tile([P, 1], F32)
            nc.sync.dma_start(cnt_sb[:], cnt_in[:])

            ident = sbp.tile([P, P], F32)
            make_identity(nc, ident[:])

            dis_exp = sbp.tile([P, TILES * HID], F32)
            de3 = bass.AP(dis_exp[:].tensor, dis_exp[:].offset,
                          [[dis_exp[:].ap[0][0], P], [HID, TILES], [1, HID]])
            db3 = bass.AP(dis_sb[:].tensor, dis_sb[:].offset,
                          [[dis_sb[:].ap[0][0], P], [1, TILES], [0, HID]])
            nc.vector.tensor_copy(out=de3, in_=db3)

            y_own = sbp.tile([P, TILES * HID], F32)
            h_sb = sbp.tile([P, TILES * HID], F32)
            slab = [sbp.tile([P, TILES * HID], F32, tag=f"slab{r}", name=f"slab{r}")
                    for r in range(4)]
            pool_slab = sbp.tile([P, HID], F32)
            zeros_sb = sbp.tile([P, HID], F32)
            nc.vector.memset(zeros_sb[:], 0.0)

            qn = [0]

            def gather(idx_tile, wcol_off, n_idx, out_ap, q, buf):
                in_ap = bass.AP(tbl[buf][:].tensor, q * HID,
                                [[4 * HID, GROWS], [1, HID]])
                _dma_gather(
                    nc.gpsimd,
                    out_ap=out_ap,
                    in_ap=in_ap,
                    idxs_ap=idx_tile[:, wcol_off:wcol_off + n_idx // 16],
                    num_idxs=n_idx,
                    num_idxs_reg=n_idx,
                    elem_size=HID,
                    elem_step=4 * HID,
                    single_packet=False,
                    queue_num=qn[0] % 4,
                )
                qn[0] += 1

            def write_ag_in(src_t):
                a = src_t[:]
                src3 = bass.AP(a.tensor, a.offset,
                               [[a.ap[0][0], P], [HID, TILES], [1, HID]])
                dst3 = bass.AP(ag_in[:].tensor, 0,
                               [[HID, P], [P * HID, TILES], [1, HID]])
                nc.sync.dma_start(dst3, src3)
                # zero the 44 pad rows (12500..12543) -- they become the
                # table's shared zero rows (PAD_IDX gathers read row 12500+q)
                zsrc = bass.AP(zeros_sb[:].tensor, zeros_sb[:].offset,
                               [[zeros_sb[:].ap[0][0], 44], [1, HID]])
                zdst = bass.AP(ag_in[:].tensor, NPC * HID,
                               [[HID, 44], [1, HID]])
                nc.sync.dma_start(zdst, zsrc)

            def epilogue_and_y(layer):
                """h = relu(dis*(S0+S1+S2+S3+y_own)+b); if layer<5 compute
                y' = dis*(h@W_{l+1}) into y_own; write ag_in."""
                s = slab[0][:]
                for r in range(1, 4):
                    nc.vector.tensor_add(out=s, in0=s, in1=slab[r][:])
                nc.vector.tensor_add(out=s, in0=s, in1=y_own[:])
                nc.vector.tensor_mul(out=s, in0=s, in1=dis_exp[:])
                boff = (layer - 1) * HID
                bb = bass.AP(b_sb[:].tensor, b_sb[:].offset + boff,
                             [[b_sb[:].ap[0][0], P], [0, TILES], [1, HID]])
                s3 = bass.AP(s.tensor, s.offset,
                             [[s.ap[0][0], P], [HID, TILES], [1, HID]])
                nc.vector.tensor_tensor(out=s3, in0=s3, in1=bb, op=AL.add)
                nc.vector.tensor_scalar(out=h_sb[:], in0=s, scalar1=0.0,
                                        scalar2=None, op0=AL.max)

                if layer < 5:
                    W = w_sb[layer + 1]
                    EB = 3
                    for b0 in range(0, NBLK, EB):
                        nb = min(EB, NBLK - b0)
                        pts, hts, pms, ws_ = [], [], [], []
                        for j in range(nb):
                            b = b0 + j
                            w = min(8, TILES - b * 8) * HID
                            ws_.append(w)
                            pt = pst.tile([P, P], F32, tag="tp", space="PSUM",
                                          name="pt")
                            pts.append(pt)
                            nc.tensor.transpose(
                                out=pt[:w, :],
                                in_=h_sb[:, b * 8 * HID:b * 8 * HID + w],
                                identity=ident[:])
                        for j in range(nb):
                            ht = smp.tile([P, P], F32, tag="ht", name="ht")
                            hts.append(ht)
                            nc.vector.tensor_copy(out=ht[:ws_[j], :],
                                                  in_=pts[j][:ws_[j], :])
                        for j in range(nb):
                            pm = psp.tile([P, P], F32, tag="mmb", space="PSUM",
                                          name="pm")
                            pms.append(pm)
                            nc.tensor.matmul(
                                out=pm[:], lhsT=hts[j][:], rhs=W[:],
                                start=True, stop=True)
                        for j in range(nb):
                            b = b0 + j
                            w = ws_[j]
                            nc.vector.tensor_mul(
                                out=y_own[:, b * 8 * HID:b * 8 * HID + w],
                                in0=pms[j][:, :w],
                                in1=dis_exp[:, b * 8 * HID:b * 8 * HID + w])
                    write_ag_in(y_own)
                else:
                    write_ag_in(h_sb)

            def allgather(buf):
                nc.gpsimd.collective_compute(
                    "AllGather", AL.bypass,
                    replica_groups=[list(range(C))],
                    ins=[ag_in[:]], outs=[tbl[buf][:]])

            def layer1_y():
                EB = 3
                for b0 in range(0, NBLK, EB):
                    nb = min(EB, NBLK - b0)
                    pms, ws_ = [], []
                    for j in range(nb):
                        b = b0 + j
                        ws_.append(min(8, TILES - b * 8) * HID)
                        pm = psp.tile([P, P], F32, tag="mmb", space="PSUM",
                                      name="pm")
                        pms.append(pm)
                        nc.tensor.matmul(
                            out=pm[:], lhsT=xt_sb[:, b * P:(b + 1) * P],
                            rhs=w_sb[1][:], start=True, stop=True)
                    for j in range(nb):
                        b = b0 + j
                        w = ws_[j]
                        nc.vector.tensor_mul(
                            out=y_own[:, b * 8 * HID:b * 8 * HID + w],
                            in0=pms[j][:, :w],
                            in1=dis_exp[:, b * 8 * HID:b * 8 * HID + w])
                write_ag_in(y_own)

            g_woff = [0]
            for r in range(4):
                g_woff.append(g_woff[-1] + int(gcols[r]) * 8)

            def reduce_buf(g, runs, sl, t0_abs, col_base_buf):
                gps = g[:].ap[0][0]
                base = g[:].offset
                for K, toff, m in runs:
                    rb = base + (col_base_buf[toff]) * HID
                    t_abs = t0_abs + toff
                    sl_ap = sl[:]
                    dst = bass.AP(sl_ap.tensor,
                                  sl_ap.offset + t_abs * HID,
                                  [[sl_ap.ap[0][0], P], [HID, m], [1, HID]])
                    if K == 1:
                        src = bass.AP(g[:].tensor, rb,
                                      [[gps, P], [HID, m], [1, HID]])
                        nc.vector.tensor_copy(out=dst, in_=src)
                        continue
                    k = K
                    while k > 2:
                        h = (k + 1) // 2
                        s = k - h
                        o = bass.AP(g[:].tensor, rb,
                                    [[gps, P], [K * HID, m], [1, s * HID]])
                        i1 = bass.AP(g[:].tensor, rb + h * HID,
                                     [[gps, P], [K * HID, m], [1, s * HID]])
                        nc.vector.tensor_tensor(out=o, in0=o, in1=i1, op=AL.add)
                        k = h
                    i0 = bass.AP(g[:].tensor, rb,
                                 [[gps, P], [K * HID, m], [1, HID]])
                    i1 = bass.AP(g[:].tensor, rb + HID,
                                 [[gps, P], [K * HID, m], [1, HID]])
                    nc.vector.tensor_tensor(out=dst, in0=i0, in1=i1, op=AL.add)

            def message_pass(buf):
                for r in range(4):
                    cb = plan["col_base"][r]
                    for (t0, ntl, col0, ncols, runs) in bufs[r]:
                        g = gp.tile([P, BUFCOLS * HID], F32, tag="g", name="g")
                        done = 0
                        while done < ncols:
                            nci = min(ICOLS, ncols - done)
                            out3 = bass.AP(g[:].tensor,
                                           g[:].offset + done * HID,
                                           [[g[:].ap[0][0], P], [HID, nci],
                                            [1, HID]])
                            gather(idx_g,
                                   g_woff[r] + (col0 + done) * 8,
                                   nci * P, out3, r, buf)
                            done += nci
                        col_base_buf = [int(cb[t0 + j] - col0)
                                        for j in range(ntl)]
                        reduce_buf(g, runs, slab[r], t0, col_base_buf)
                # slabs -> rslab -> align gather back into slab[r]
                for r in range(4):
                    sl = slab[r][:]
                    src3 = bass.AP(sl.tensor, sl.offset,
                                   [[sl.ap[0][0], P], [HID, TILES], [1, HID]])
                    dst3 = bass.AP(rslab[:].tensor, r * NPAD * ROW,
                                   [[ROW, P], [P * ROW, TILES], [1, HID]])
                    nc.sync.dma_start(dst3, src3)
                for r in range(4):
                    awoff = r * (P * 8)
                    done = 0
                    while done < TILES:
                        ntl = min(ICOLS, TILES - done)
                        n_idx = ntl * P
                        sl = slab[r][:]
                        out3 = bass.AP(sl.tensor, sl.offset + done * HID,
                                       [[sl.ap[0][0], P], [HID, ntl], [1, HID]])
                        in_ap = bass.AP(rslab[:].tensor, r * NPAD * ROW,
                                        [[ROW, NPAD], [1, HID]])
                        _dma_gather(
                            nc.gpsimd, out_ap=out3, in_ap=in_ap,
                            idxs_ap=idx_a[:, awoff + done * 8:
                                          awoff + (done + ntl) * 8],
                            num_idxs=n_idx, num_idxs_reg=n_idx,
                            elem_size=HID, elem_step=ROW,
                            single_packet=False, queue_num=qn[0] % 4)
                        qn[0] += 1
                        done += ntl

            def pooling_and_head(buf):
                woff = 0
                first = True
                for r in range(4):
                    K = Kp[r]
                    g = gp.tile([P, BUFCOLS * HID], F32, tag="g", name="g")
                    done = 0
                    while done < K:
                        nci = min(ICOLS, K - done)
                        out3 = bass.AP(g[:].tensor, g[:].offset + done * HID,
                                       [[g[:].ap[0][0], P], [HID, nci], [1, HID]])
                        gather(idx_p, woff + done * 8, nci * P, out3, r, buf)
                        done += nci
                    woff += K * 8
                    k = K
                    while k > 1:
                        h = (k + 1) // 2
                        srcn = k - h
                        nc.vector.tensor_add(
                            out=g[:, :srcn * HID], in0=g[:, :srcn * HID],
                            in1=g[:, h * HID:(h + srcn) * HID])
                        k = h
                    if first:
                        nc.vector.tensor_copy(out=pool_slab[:], in_=g[:, :HID])
                        first = False
                    else:
                        nc.vector.tensor_add(out=pool_slab[:], in0=pool_slab[:],
                                             in1=g[:, :HID])
                rcp = smp.tile([P, 1], F32, tag="rcp")
                nc.vector.reciprocal(out=rcp[:], in_=cnt_sb[:])
                nc.vector.tensor_scalar(out=pool_slab[:], in0=pool_slab[:],
                                        scalar1=rcp[:], scalar2=None,
                                        op0=AL.mult)

                def rrelu(ap):
                    pos = smp.tile([P, HID], F32, tag="rr1")
                    nc.vector.tensor_scalar(out=pos[:, :ap.shape[1]], in0=ap,
                                            scalar1=0.0, scalar2=None, op0=AL.max)
                    nc.vector.tensor_scalar(out=ap, in0=ap, scalar1=0.0,
                                            scalar2=RRELU_SLOPE, op0=AL.min,
                                            op1=AL.mult)
                    nc.vector.tensor_add(out=ap, in0=ap,
                                         in1=pos[:, :ap.shape[1]])

                pt = pst.tile([P, P], F32, tag="tp", space="PSUM")
                nc.tensor.transpose(out=pt[:HID, :], in_=pool_slab[:],
                                    identity=ident[:])
                gt = smp.tile([HID, P], F32, tag="gt")
                nc.vector.tensor_copy(out=gt[:], in_=pt[:HID, :])
                pm = pst.tile([P, HID], F32, tag="tp", space="PSUM", name="pmp")
                nc.tensor.matmul(out=pm[:], lhsT=gt[:], rhs=l1w_sb[:],
                                 start=True, stop=True)
                g1 = smp.tile([P, HID], F32, tag="g1")
                nc.vector.tensor_add(out=g1[:], in0=pm[:], in1=l1b_sb[:])
                rrelu(g1[:])
                pt2 = pst.tile([P, P], F32, tag="tp", space="PSUM")
                nc.tensor.transpose(out=pt2[:HID, :], in_=g1[:],
                                    identity=ident[:])
                gt2 = smp.tile([HID, P], F32, tag="gt")
                nc.vector.tensor_copy(out=gt2[:], in_=pt2[:HID, :])
                pm2 = pst.tile([P, 1], F32, tag="tp", space="PSUM", name="pmp2")
                nc.tensor.matmul(out=pm2[:], lhsT=gt2[:], rhs=l2w_sb[:],
                                 start=True, stop=True)
                g2 = smp.tile([P, 1], F32, tag="g2")
                nc.vector.tensor_add(out=g2[:], in0=pm2[:], in1=l2b_sb[:])
                rrelu(g2[:])
                nc.sync.dma_start(out_t[:], g2[:])

            if mode == "full":
                for rep in range(reps):
                    sc = (lambda name: nc.named_scope(f"r{rep}_{name}")) if reps > 1 else nc.named_scope
                    with sc("l1"):
                        layer1_y()
                    with sc("ag0"):
                        allgather(0)
                    for layer in range(1, 6):
                        with sc(f"mp{layer}"):
                            message_pass((layer - 1) % 2)
                        with sc(f"ep{layer}"):
                            epilogue_and_y(layer)
                        with sc(f"ag{layer}"):
                            allgather(layer % 2)
                    with sc("pool"):
                        pooling_and_head(5 % 2)

    nc.finalize()
    return nc


def _make_in_maps(per_core, inputs):
    W1, W2, W3, W4, W5 = (inputs[f"W{i}"] for i in range(1, 6))
    bs = np.concatenate([np.asarray(inputs[f"b{i}"], np.float32)
                         for i in range(1, 6)]).reshape(1, 5 * HID)
    bs = np.repeat(bs, P, axis=0).copy()
    l1b = np.repeat(np.asarray(inputs["lin1_b"], np.float32).reshape(1, HID), P, 0).copy()
    l2b = np.repeat(np.asarray(inputs["lin2_b"], np.float32).reshape(1, 1), P, 0).copy()

    in_maps = []
    for c in range(C):
        pc = per_core[c]
        in_maps.append({
            "deg_tiles": pc["deg_tiles"].astype(np.float32),
            "xt": pc["xt"],
            "gather_w": pc["gather_w"],
            "align_w": pc["align_w"],
            "pool_w": pc["pool_w"],
            "cnt": pc["cnt"],
            "W1": np.kron(np.eye(8, dtype=np.float32), np.asarray(W1, np.float32)),
            "W2": np.kron(np.eye(8, dtype=np.float32), np.asarray(W2, np.float32)),
            "W3": np.kron(np.eye(8, dtype=np.float32), np.asarray(W3, np.float32)),
            "W4": np.kron(np.eye(8, dtype=np.float32), np.asarray(W4, np.float32)),
            "W5": np.kron(np.eye(8, dtype=np.float32), np.asarray(W5, np.float32)),
            "bs": bs,
            "lin1_w": np.asarray(inputs["lin1_w"], np.float32),
            "lin1_b": l1b,
            "lin2_w": np.asarray(inputs["lin2_w"], np.float32),
            "lin2_b": l2b,
        })
    return in_maps


def kernel(x, edge_index, batch, W1, b1, W2, b2, W3, b3, W4, b4, W5, b5,
           lin1_w, lin1_b, lin2_w, lin2_b, _reps=1, _prebuilt=None):
    per_core, plan = _preprocess(x, edge_index, batch)
    nc = _prebuilt if _prebuilt is not None else _build_program(plan, reps=_reps)
    inputs = dict(x=x, edge_index=edge_index, batch=batch, W1=W1, b1=b1, W2=W2,
                  b2=b2, W3=W3, b3=b3, W4=W4, b4=b4, W5=W5, b5=b5,
                  lin1_w=lin1_w, lin1_b=lin1_b, lin2_w=lin2_w, lin2_b=lin2_b)
    in_maps = _make_in_maps(per_core, inputs)

    res = run_bass_via_pjrt(nc, in_maps, n_cores=C)
    out = np.zeros((N_GRAPHS, 1), dtype=np.float32)
    for c in range(C):
        out[c * GPC:(c + 1) * GPC, 0] = res[c]["out"][:GPC, 0]
    return out
